# revision 19
# baseline (speedup 1.0000x reference)
# Trainium2 Bass kernel for nn_AblatedPairEnergies (gnn_message_passing).
#
# reference semantics:
#   E_idx = top-30 nearest neighbors by masked CA-atom distance  [B,L,K] int32
#   h_EV  = concat(V[E_idx[:,:,0]], V[E_idx]) @ W_w + W_b        [B,L,K,400] f32
#
# Device strategy (per core; 8 cores = 4 batches x 2 L-halves):
#   1. s[l,j] = ((dx^2+dy^2)+dz^2) computed elementwise so it is bit-identical
#      (mod the constant +1e-6 shift) to the CPU jax reference's pre-sqrt value.
#   2. top-32 candidates per row via 4 rounds of DVE max8/max_index/
#      match_replace on -s.
#   3. h_EV[l,k] = P1b[E_idx[l,0]] + P2[E_idx[l,k]] where P1b = V@W_w[:128]+W_b
#      and P2 = V@W_w[128:] (algebraic split of the concat-matmul). P1b/P2
#      (rows padded to 448 f32 for the 256B-multiple gather constraint) are
#      computed once per core on the PE and staged to DRAM in an interleaved
#      row order (row j at slot (j%128)*16 + j//128) so the stores are
#      one-descriptor-per-partition; gather indices are remapped on device.
#      Rows come back via dma_gather in 1024-descriptor chunks (the SWDGE
#      ring holds ~1024). dma_gather's int16 [16, n/16]-wrapped index layout
#      (replicated across all 8 16-partition groups) is built fully on-chip
#      with two rounds of PE transposes; the broadcast add runs on DVE.
#      All 32 candidate rows are returned.
#   4. host epilogue: recompute the 32 selected distances bit-exactly in fp32
#      (incl. IEEE sqrt) and stable-sort each row's candidates by (D, index) --
#      jax.lax.top_k's exact tie-break -- then keep the first 30. This O(B*L*32)
#      fixup resolves fp32 sqrt-collapse ties (distinct s rounding to the same
#      sqrt) which value-ordered device selection cannot see; the ~GFLOP /
#      ~400 MB heavy lifting all happens on device.
#
# x_mask is all-ones for this problem (spec fill=ones), which makes the
# reference's masking a no-op; the kernel relies on that.

import numpy as np

B = 4
L = 2048
K = 30
K2 = 32  # candidates selected on device
C = 128
OUT = 400
PADOUT = 448  # gather element must be a multiple of 256B -> 448 f32 = 1792B
N_CORES = 8
ROWS_PER_CORE = L // 2  # 1024
P = 128  # partitions / rows per tile
NEG_BIG = -3.0e38

_CACHE = {}


def _build_nc(n_tiles=ROWS_PER_CORE // P):
    from concourse import bacc, mybir, tile
    import concourse.bass as bass
    from concourse.masks import make_identity

    f32 = mybir.dt.float32
    u32 = mybir.dt.uint32
    i16 = mybir.dt.int16
    rows = n_tiles * P
    WCOL = K2 * P // 16  # wrapped index columns (256)
    NCH = L // P         # projection row-chunks (16)

    nc = bacc.Bacc("TRN2", target_bir_lowering=False, num_swdge_queues=4)

    # --- DRAM I/O ---
    xjt = nc.dram_tensor("xjt", [3, L], f32, kind="ExternalInput")       # CA coords^T, whole batch
    xl = nc.dram_tensor("xl", [rows, 3], f32, kind="ExternalInput")      # own rows' CA coords
    vt = nc.dram_tensor("vt", [C, L], f32, kind="ExternalInput")         # V_embed^T, whole batch
    w1 = nc.dram_tensor("w1", [C, OUT], f32, kind="ExternalInput")
    w2 = nc.dram_tensor("w2", [C, OUT], f32, kind="ExternalInput")
    wb = nc.dram_tensor("wb", [1, OUT], f32, kind="ExternalInput")
    hev = nc.dram_tensor("hev", [rows, K2 * OUT], f32, kind="ExternalOutput")
    eidx = nc.dram_tensor("eidx", [rows, K2], u32, kind="ExternalOutput")

    # combined interleaved projection table: row (j%128)*16 + j//128 holds
    # P2[j]; row L + (j%128)*16 + j//128 holds P1b[j] = V@W1 + Wb
    t_d = nc.dram_tensor("t_d", [2 * L, PADOUT], f32)

    with tile.TileContext(nc) as tc:
        with (
            tc.tile_pool(name="const", bufs=1) as const,
            tc.tile_pool(name="sq", bufs=2) as sqp,
            tc.tile_pool(name="sel", bufs=2) as sel,
            tc.tile_pool(name="tpsum", bufs=2, space="PSUM") as tpsum,
        ):
            # ---- constants ----
            xbc = const.tile([P, 3, L], f32)      # candidate coords broadcast to all partitions
            for c in range(3):
                nc.sync.dma_start(out=xbc[:, c, :], in_=xjt[c : c + 1, :].partition_broadcast(P))
            # weights padded to PADOUT with zeros so P1b/P2 rows are fully defined
            w1_sb = const.tile([C, PADOUT], f32)
            nc.vector.memset(w1_sb[:, OUT:], 0.0)
            nc.sync.dma_start(out=w1_sb[:, :OUT], in_=w1[:, :])
            w2_sb = const.tile([C, PADOUT], f32)
            nc.vector.memset(w2_sb[:, OUT:], 0.0)
            nc.sync.dma_start(out=w2_sb[:, :OUT], in_=w2[:, :])
            wb_bc = const.tile([P, PADOUT], f32)
            nc.vector.memset(wb_bc[:, OUT:], 0.0)
            nc.sync.dma_start(out=wb_bc[:, :OUT], in_=wb[0:1, :].partition_broadcast(P))
            xl_all = const.tile([P, n_tiles, 3], f32)
            nc.sync.dma_start(
                out=xl_all[:, :, :],
                in_=xl[:, :].rearrange("(t p) c -> p t c", p=P, t=n_tiles),
            )
            neg_xl = const.tile([P, n_tiles, 3], f32)
            nc.vector.tensor_scalar_mul(neg_xl[:, :, :], xl_all[:, :, :], -1.0)
            ident = const.tile([P, P], f32)
            make_identity(nc, ident[:, :])
            eidx_all = const.tile([P, n_tiles, K2], u32)

            # ---- P1b = V@W1 + Wb, P2 = V@W2 (whole batch), staged to DRAM ----
            with (
                tc.tile_pool(name="mm", bufs=1) as mmp,
            ):
                vt_sb = mmp.tile([C, L], f32)
                nc.sync.dma_start(out=vt_sb[:, :], in_=vt[:, :])
                p1_all = mmp.tile([P, NCH, PADOUT], f32)
                p2_all = mmp.tile([P, NCH, PADOUT], f32)
                for i in range(NCH):
                    vslice = vt_sb[:, i * P : (i + 1) * P]
                    ps1 = tpsum.tile([P, PADOUT], f32, tag="ps1", space="PSUM")
                    nc.tensor.matmul(out=ps1[:, :], lhsT=vslice, rhs=w1_sb[:, :], start=True, stop=True)
                    nc.vector.tensor_add(out=p1_all[:, i, :], in0=ps1[:, :], in1=wb_bc[:, :])
                    ps2 = tpsum.tile([P, PADOUT], f32, tag="ps2", space="PSUM")
                    nc.tensor.matmul(out=ps2[:, :], lhsT=vslice, rhs=w2_sb[:, :], start=True, stop=True)
                    nc.scalar.copy(out=p2_all[:, i, :], in_=ps2[:, :])
                # one store each; physical row (j%128)*16 + j//128 makes these
                # a single contiguous 28.7KB descriptor per partition
                nc.scalar.dma_start(
                    out=t_d[:L, :].rearrange("(p c) o -> p c o", p=P, c=NCH),
                    in_=p2_all[:, :, :],
                )
                nc.sync.dma_start(
                    out=t_d[L:, :].rearrange("(p c) o -> p c o", p=P, c=NCH),
                    in_=p1_all[:, :, :],
                )

            # ---- per l-tile pipeline ----
            gatp_cm = tc.tile_pool(name="gat", bufs=2)
            gatp = gatp_cm.__enter__()
            gcount = 0
            for t in range(n_tiles):
                # s = ((dx^2 + dy^2) + dz^2); selection key = -s
                s = sqp.tile([P, L], f32, tag="s")
                nc.scalar.activation(
                    out=s[:, :], in_=xbc[:, 0, :],
                    func=mybir.ActivationFunctionType.Square,
                    bias=neg_xl[:, t, 0:1], scale=1.0,
                )
                for c in (1, 2):
                    sq = sqp.tile([P, L], f32, tag="sqc")
                    nc.scalar.activation(
                        out=sq[:, :], in_=xbc[:, c, :],
                        func=mybir.ActivationFunctionType.Square,
                        bias=neg_xl[:, t, c : c + 1], scale=1.0,
                    )
                    nc.vector.tensor_add(out=s[:, :], in0=s[:, :], in1=sq[:, :])
                negs = s
                nc.vector.tensor_scalar_mul(negs[:, :], s[:, :], -1.0)

                # top-32 candidates: 4 rounds of max8 + max_index + match_replace
                vals = sel.tile([P, K2], f32, tag="vals")
                idxs = sel.tile([P, K2], u32, tag="idxs")
                for r in range(4):
                    sl = slice(r * 8, (r + 1) * 8)
                    nc.vector.max(out=vals[:, sl], in_=negs[:, :])
                    nc.vector.max_index(out=idxs[:, sl], in_max=vals[:, sl], in_values=negs[:, :])
                    if r < 3:
                        nc.vector.match_replace(
                            out=negs[:, :], in_to_replace=vals[:, sl],
                            in_values=negs[:, :], imm_value=NEG_BIG,
                        )
                nc.vector.tensor_copy(out=eidx_all[:, t, :], in_=idxs[:, :])

                # remap indices to the interleaved table order:
                # row(j) = (j & 127)*16 + (j >> 7)
                rowt = sel.tile([P, K2 + 1], u32, tag="rowt")
                rsh = sel.tile([P, K2], u32, tag="rsh")
                nc.vector.tensor_scalar(
                    out=rowt[:, :K2], in0=idxs[:, :], scalar1=127, scalar2=4,
                    op0=mybir.AluOpType.bitwise_and,
                    op1=mybir.AluOpType.logical_shift_left,
                )
                nc.vector.tensor_scalar(
                    out=rsh[:, :], in0=idxs[:, :], scalar1=7, scalar2=None,
                    op0=mybir.AluOpType.logical_shift_right,
                )
                nc.vector.tensor_tensor(
                    out=rowt[:, :K2], in0=rowt[:, :K2], in1=rsh[:, :],
                    op=mybir.AluOpType.bitwise_or,
                )
                nc.vector.tensor_scalar_add(rowt[:, K2 : K2 + 1], rowt[:, 0:1], float(L))

                # build dma_gather's wrapped int16 index layout on-chip:
                # wrapped[16g+q, k*8+r] = row(E[16r+q, k]) for all groups g.
                K3 = K2 + 1
                rowf = sel.tile([P, K3], f32, tag="rowf")
                nc.vector.tensor_copy(out=rowf[:, :], in_=rowt[:, :])
                mtp = tpsum.tile([K3, P], f32, tag="mtp", space="PSUM")
                nc.tensor.transpose(out=mtp[:, :], in_=rowf[:, :], identity=ident[:, :])
                mt_sb = sel.tile([K3, P], f32, tag="mt_sb")
                nc.vector.tensor_copy(out=mt_sb[:, :], in_=mtp[:, :])
                wrapped = sel.tile([P, K3 * 8], i16, tag="wrapped")
                wview = wrapped[:, :].rearrange("p (k r) -> p k r", k=K3, r=8)
                for r in range(8):
                    zr = tpsum.tile([16, K3], f32, tag="zr", space="PSUM")
                    nc.tensor.transpose(
                        out=zr[:, :], in_=mt_sb[:, 16 * r : 16 * (r + 1)],
                        identity=ident[0:K3, 0:K3],
                    )
                    nc.vector.tensor_copy(out=wview[0:16, :, r], in_=zr[:, :])
                for g in range(1, 8):
                    eng = nc.scalar if g % 2 else nc.sync
                    eng.dma_start(out=wrapped[16 * g : 16 * (g + 1), :], in_=wrapped[0:16, :])

                # gather P2 rows for k=0..31 plus P1b[e0] as chunk 32,
                # all from the combined table, <=1024 descriptors per call
                gat = gatp.tile([P, K3, PADOUT], f32, tag="gat")
                k0 = 0
                for kc in (7, 7, 7, 6, 6):
                    nc.gpsimd.dma_gather(
                        out_ap=gat[:, k0 : k0 + kc, :], in_ap=t_d[:, :],
                        idxs_ap=wrapped[:, k0 * 8 : (k0 + kc) * 8],
                        num_idxs=kc * P, num_idxs_reg=kc * P, elem_size=PADOUT,
                        queue_num=gcount % 4,
                    )
                    gcount += 1
                    k0 += kc
                SPL = 20  # chunks added on DVE; rest on GpSimd
                nc.vector.tensor_add(
                    out=gat[:, :SPL, :], in0=gat[:, :SPL, :],
                    in1=gat[:, K2 : K2 + 1, :].to_broadcast([P, SPL, PADOUT]),
                )
                nc.gpsimd.tensor_add(
                    out=gat[:, SPL:K2, :], in0=gat[:, SPL:K2, :],
                    in1=gat[:, K2 : K2 + 1, :].to_broadcast([P, K2 - SPL, PADOUT]),
                )
                nc.sync.dma_start(
                    out=hev[t * P : (t + 1) * P, :],
                    in_=gat[:, :K2, :OUT],
                )
            gatp_cm.__exit__(None, None, None)

            nc.scalar.dma_start(
                out=eidx[:, :].rearrange("(t p) k -> p t k", t=n_tiles, p=P),
                in_=eidx_all[:, :, :],
            )

    nc.compile()
    return nc


def _prepare_in_maps(X, V_embed, W_w, W_b):
    X = np.asarray(X, dtype=np.float32)
    V_embed = np.asarray(V_embed, dtype=np.float32)
    W_w = np.asarray(W_w, dtype=np.float32)
    W_b = np.asarray(W_b, dtype=np.float32)

    Xca = X[:, :, 1, :]  # [B, L, 3]
    in_maps = []
    for core in range(N_CORES):
        b, h = divmod(core, 2)
        in_maps.append({
            "xjt": np.ascontiguousarray(Xca[b].T),
            "xl": np.ascontiguousarray(Xca[b, h * ROWS_PER_CORE : (h + 1) * ROWS_PER_CORE]),
            "vt": np.ascontiguousarray(V_embed[b].T),
            "w1": np.ascontiguousarray(W_w[:C]),
            "w2": np.ascontiguousarray(W_w[C:]),
            "wb": np.ascontiguousarray(W_b.reshape(1, OUT)),
        })
    return in_maps


def _host_order_fixup(X, h32, e32):
    # Reorder each row's 32 candidates by (fp32 D, index) -- jax.lax.top_k's
    # exact ordering incl. sqrt-collapse ties -- and keep the first K.
    # D is recomputed bit-exactly: same elementwise fp32 ops as the reference.
    Xca = np.asarray(X, dtype=np.float32)[:, :, 1, :]  # [B, L, 3]
    h_EV = np.empty((B, L, K, OUT), np.float32)
    E_idx = np.empty((B, L, K), np.int32)
    for b in range(B):
        a = Xca[b]                      # [L, 3]
        cpts = Xca[b][e32[b]]           # [L, 32, 3]
        dx = (a[:, None, 0] - cpts[:, :, 0]).astype(np.float32)
        dy = (a[:, None, 1] - cpts[:, :, 1]).astype(np.float32)
        dz = (a[:, None, 2] - cpts[:, :, 2]).astype(np.float32)
        s = ((dx * dx + dy * dy) + dz * dz) + np.float32(1e-6)
        D = np.sqrt(s, dtype=np.float32)
        perm = np.lexsort((e32[b], D), axis=1)[:, :K]   # [L, K]
        E_idx[b] = np.take_along_axis(e32[b], perm, axis=1)
        h_EV[b] = np.take_along_axis(h32[b], perm[:, :, None], axis=1)
    return h_EV, E_idx


def _run(X, x_mask, V_embed, W_w, W_b, **run_kwargs):
    from concourse.bass_utils import run_bass_kernel_spmd

    if "nc" not in _CACHE:
        _CACHE["nc"] = _build_nc()
    nc = _CACHE["nc"]

    in_maps = _prepare_in_maps(X, V_embed, W_w, W_b)
    bkr = run_bass_kernel_spmd(nc, in_maps, list(range(N_CORES)), **run_kwargs)
    res = bkr.results

    h32 = np.empty((B, L, K2, OUT), np.float32)
    e32 = np.empty((B, L, K2), np.int64)
    for core in range(N_CORES):
        b, h = divmod(core, 2)
        rows = slice(h * ROWS_PER_CORE, (h + 1) * ROWS_PER_CORE)
        h32[b, rows] = res[core]["hev"].reshape(ROWS_PER_CORE, K2, OUT)
        e32[b, rows] = res[core]["eidx"].astype(np.int64)
    h_EV, E_idx = _host_order_fixup(X, h32, e32)
    return h_EV, E_idx, bkr


def kernel(X, x_mask, V_embed, W_w, W_b):
    h_EV, E_idx, _ = _run(X, x_mask, V_embed, W_w, W_b)
    return h_EV, E_idx


# revision 20
# speedup vs baseline: 1.2649x; 1.2649x over previous
# Trainium2 Bass kernel for nn_AblatedPairEnergies (gnn_message_passing).
#
# reference semantics:
#   E_idx = top-30 nearest neighbors by masked CA-atom distance  [B,L,K] int32
#   h_EV  = concat(V[E_idx[:,:,0]], V[E_idx]) @ W_w + W_b        [B,L,K,400] f32
#
# Device strategy (per core; 8 cores = 4 batches x 2 L-halves):
#   1. s[l,j] = ((dx^2+dy^2)+dz^2) computed elementwise so it is bit-identical
#      (mod the constant +1e-6 shift) to the CPU jax reference's pre-sqrt value.
#   2. top-32 candidates per row via 4 rounds of DVE max8/max_index/
#      match_replace on -s.
#   3. h_EV[l,k] = P1b[E_idx[l,0]] + P2[E_idx[l,k]] where P1b = V@W_w[:128]+W_b
#      and P2 = V@W_w[128:] (algebraic split of the concat-matmul). P1b/P2
#      (rows padded to 448 f32 for the 256B-multiple gather constraint) are
#      computed once per core on the PE and staged to DRAM in an interleaved
#      row order (row j at slot (j%128)*16 + j//128) so the stores are
#      one-descriptor-per-partition; gather indices are remapped on device.
#      Rows come back via dma_gather in 1024-descriptor chunks (the SWDGE
#      ring holds ~1024). dma_gather's int16 [16, n/16]-wrapped index layout
#      (replicated across all 8 16-partition groups) is built fully on-chip
#      with two rounds of PE transposes; the broadcast add runs on DVE.
#      All 32 candidate rows are returned.
#   4. host epilogue: recompute the 32 selected distances bit-exactly in fp32
#      (incl. IEEE sqrt) and stable-sort each row's candidates by (D, index) --
#      jax.lax.top_k's exact tie-break -- then keep the first 30. This O(B*L*32)
#      fixup resolves fp32 sqrt-collapse ties (distinct s rounding to the same
#      sqrt) which value-ordered device selection cannot see; the ~GFLOP /
#      ~400 MB heavy lifting all happens on device.
#
# x_mask is all-ones for this problem (spec fill=ones), which makes the
# reference's masking a no-op; the kernel relies on that.

import numpy as np

B = 4
L = 2048
K = 30
K2 = 32  # candidates selected on device
C = 128
OUT = 400
PADOUT = 448  # gather element must be a multiple of 256B -> 448 f32 = 1792B
N_CORES = 8
ROWS_PER_CORE = L // 2  # 1024
P = 128  # partitions / rows per tile
NEG_BIG = -3.0e38

_CACHE = {}


def _build_nc(n_tiles=ROWS_PER_CORE // P):
    from concourse import bacc, mybir, tile
    import concourse.bass as bass
    from concourse.masks import make_identity

    f32 = mybir.dt.float32
    u32 = mybir.dt.uint32
    i16 = mybir.dt.int16
    rows = n_tiles * P
    WCOL = K2 * P // 16  # wrapped index columns (256)
    NCH = L // P         # projection row-chunks (16)

    nc = bacc.Bacc("TRN2", target_bir_lowering=False, num_swdge_queues=4)

    # --- DRAM I/O ---
    xjt = nc.dram_tensor("xjt", [3, L], f32, kind="ExternalInput")       # CA coords^T, whole batch
    xl = nc.dram_tensor("xl", [rows, 3], f32, kind="ExternalInput")      # own rows' CA coords
    vt = nc.dram_tensor("vt", [C, L], f32, kind="ExternalInput")         # V_embed^T, whole batch
    w1 = nc.dram_tensor("w1", [C, OUT], f32, kind="ExternalInput")
    w2 = nc.dram_tensor("w2", [C, OUT], f32, kind="ExternalInput")
    wb = nc.dram_tensor("wb", [1, OUT], f32, kind="ExternalInput")
    hev = nc.dram_tensor("hev", [rows, K2 * OUT], f32, kind="ExternalOutput")
    eidx = nc.dram_tensor("eidx", [rows, K2], u32, kind="ExternalOutput")

    # combined interleaved projection table: row (j%128)*16 + j//128 holds
    # P2[j]; row L + (j%128)*16 + j//128 holds P1b[j] = V@W1 + Wb
    t_d = nc.dram_tensor("t_d", [2 * L, PADOUT], f32)

    with tile.TileContext(nc) as tc:
        with (
            tc.tile_pool(name="const", bufs=1) as const,
            tc.tile_pool(name="sq", bufs=2) as sqp,
            tc.tile_pool(name="sel", bufs=2) as sel,
            tc.tile_pool(name="tpsum", bufs=2, space="PSUM") as tpsum,
        ):
            # ---- constants ----
            xbc = const.tile([P, 3, L], f32)      # candidate coords broadcast to all partitions
            for c in range(3):
                nc.sync.dma_start(out=xbc[:, c, :], in_=xjt[c : c + 1, :].partition_broadcast(P))
            # weights padded to PADOUT with zeros so P1b/P2 rows are fully defined
            w1_sb = const.tile([C, PADOUT], f32)
            nc.vector.memset(w1_sb[:, OUT:], 0.0)
            nc.sync.dma_start(out=w1_sb[:, :OUT], in_=w1[:, :])
            w2_sb = const.tile([C, PADOUT], f32)
            nc.vector.memset(w2_sb[:, OUT:], 0.0)
            nc.sync.dma_start(out=w2_sb[:, :OUT], in_=w2[:, :])
            wb_bc = const.tile([P, PADOUT], f32)
            nc.vector.memset(wb_bc[:, OUT:], 0.0)
            nc.sync.dma_start(out=wb_bc[:, :OUT], in_=wb[0:1, :].partition_broadcast(P))
            xl_all = const.tile([P, n_tiles, 3], f32)
            nc.sync.dma_start(
                out=xl_all[:, :, :],
                in_=xl[:, :].rearrange("(t p) c -> p t c", p=P, t=n_tiles),
            )
            neg_xl = const.tile([P, n_tiles, 3], f32)
            nc.vector.tensor_scalar_mul(neg_xl[:, :, :], xl_all[:, :, :], -1.0)
            ident = const.tile([P, P], f32)
            make_identity(nc, ident[:, :])
            eidx_all = const.tile([P, n_tiles, K2], u32)

            # ---- P1b = V@W1 + Wb, P2 = V@W2 (whole batch), staged to DRAM ----
            with (
                tc.tile_pool(name="mm", bufs=1) as mmp,
            ):
                vt_sb = mmp.tile([C, L], f32)
                nc.sync.dma_start(out=vt_sb[:, :], in_=vt[:, :])
                p1_all = mmp.tile([P, NCH, PADOUT], f32)
                p2_all = mmp.tile([P, NCH, PADOUT], f32)
                for i in range(NCH):
                    vslice = vt_sb[:, i * P : (i + 1) * P]
                    ps1 = tpsum.tile([P, PADOUT], f32, tag="ps1", space="PSUM")
                    nc.tensor.matmul(out=ps1[:, :], lhsT=vslice, rhs=w1_sb[:, :], start=True, stop=True)
                    nc.vector.tensor_add(out=p1_all[:, i, :], in0=ps1[:, :], in1=wb_bc[:, :])
                    ps2 = tpsum.tile([P, PADOUT], f32, tag="ps2", space="PSUM")
                    nc.tensor.matmul(out=ps2[:, :], lhsT=vslice, rhs=w2_sb[:, :], start=True, stop=True)
                    nc.scalar.copy(out=p2_all[:, i, :], in_=ps2[:, :])
                # one store each; physical row (j%128)*16 + j//128 makes these
                # a single contiguous 28.7KB descriptor per partition
                nc.scalar.dma_start(
                    out=t_d[:L, :].rearrange("(p c) o -> p c o", p=P, c=NCH),
                    in_=p2_all[:, :, :],
                )
                nc.sync.dma_start(
                    out=t_d[L:, :].rearrange("(p c) o -> p c o", p=P, c=NCH),
                    in_=p1_all[:, :, :],
                )

            # ---- per l-tile pipeline ----
            gatp_cm = tc.tile_pool(name="gat", bufs=2)
            gatp = gatp_cm.__enter__()
            gcount = 0
            for t in range(n_tiles):
                # s = ((dx^2 + dy^2) + dz^2); selection key = -s
                s = sqp.tile([P, L], f32, tag="s")
                nc.scalar.activation(
                    out=s[:, :], in_=xbc[:, 0, :],
                    func=mybir.ActivationFunctionType.Square,
                    bias=neg_xl[:, t, 0:1], scale=1.0,
                )
                for c in (1, 2):
                    sq = sqp.tile([P, L], f32, tag="sqc")
                    nc.scalar.activation(
                        out=sq[:, :], in_=xbc[:, c, :],
                        func=mybir.ActivationFunctionType.Square,
                        bias=neg_xl[:, t, c : c + 1], scale=1.0,
                    )
                    nc.vector.tensor_add(out=s[:, :], in0=s[:, :], in1=sq[:, :])
                negs = s
                nc.vector.tensor_scalar_mul(negs[:, :], s[:, :], -1.0)

                # top-32 candidates: 4 rounds of max8 + max_index + match_replace
                vals = sel.tile([P, K2], f32, tag="vals")
                idxs = sel.tile([P, K2], u32, tag="idxs")
                for r in range(4):
                    sl = slice(r * 8, (r + 1) * 8)
                    nc.vector.max(out=vals[:, sl], in_=negs[:, :])
                    nc.vector.max_index(out=idxs[:, sl], in_max=vals[:, sl], in_values=negs[:, :])
                    if r < 3:
                        nc.vector.match_replace(
                            out=negs[:, :], in_to_replace=vals[:, sl],
                            in_values=negs[:, :], imm_value=NEG_BIG,
                        )
                nc.vector.tensor_copy(out=eidx_all[:, t, :], in_=idxs[:, :])

                # remap indices to the interleaved table order:
                # row(j) = (j & 127)*16 + (j >> 7)
                rowt = sel.tile([P, K2 + 1], u32, tag="rowt")
                rsh = sel.tile([P, K2], u32, tag="rsh")
                nc.vector.tensor_scalar(
                    out=rowt[:, :K2], in0=idxs[:, :], scalar1=127, scalar2=4,
                    op0=mybir.AluOpType.bitwise_and,
                    op1=mybir.AluOpType.logical_shift_left,
                )
                nc.vector.tensor_scalar(
                    out=rsh[:, :], in0=idxs[:, :], scalar1=7, scalar2=None,
                    op0=mybir.AluOpType.logical_shift_right,
                )
                nc.vector.tensor_tensor(
                    out=rowt[:, :K2], in0=rowt[:, :K2], in1=rsh[:, :],
                    op=mybir.AluOpType.bitwise_or,
                )
                nc.vector.tensor_scalar_add(rowt[:, K2 : K2 + 1], rowt[:, 0:1], float(L))

                # build dma_gather's wrapped int16 index layout on-chip:
                # wrapped[16g+q, k*8+r] = row(E[16r+q, k]) for all groups g.
                K3 = K2 + 1
                rowf = sel.tile([P, K3], f32, tag="rowf")
                nc.vector.tensor_copy(out=rowf[:, :], in_=rowt[:, :])
                mtp = tpsum.tile([K3, P], f32, tag="mtp", space="PSUM")
                nc.tensor.transpose(out=mtp[:, :], in_=rowf[:, :], identity=ident[:, :])
                mt_sb = sel.tile([K3, P], f32, tag="mt_sb")
                nc.vector.tensor_copy(out=mt_sb[:, :], in_=mtp[:, :])
                wrapped = sel.tile([P, K3 * 8], i16, tag="wrapped")
                wview = wrapped[:, :].rearrange("p (k r) -> p k r", k=K3, r=8)
                for r in range(8):
                    zr = tpsum.tile([16, K3], f32, tag="zr", space="PSUM")
                    nc.tensor.transpose(
                        out=zr[:, :], in_=mt_sb[:, 16 * r : 16 * (r + 1)],
                        identity=ident[0:K3, 0:K3],
                    )
                    nc.vector.tensor_copy(out=wview[0:16, :, r], in_=zr[:, :])
                for g in range(1, 8):
                    eng = nc.scalar if g % 2 else nc.sync
                    eng.dma_start(out=wrapped[16 * g : 16 * (g + 1), :], in_=wrapped[0:16, :])

                # gather P2 rows for k=0..31 plus P1b[e0] as chunk 32,
                # all from the combined table, <=1024 descriptors per call
                gat = gatp.tile([P, K3, PADOUT], f32, tag="gat")
                k0 = 0
                for kc in (7, 7, 7, 6, 6):
                    nc.gpsimd.dma_gather(
                        out_ap=gat[:, k0 : k0 + kc, :], in_ap=t_d[:, :],
                        idxs_ap=wrapped[:, k0 * 8 : (k0 + kc) * 8],
                        num_idxs=kc * P, num_idxs_reg=kc * P, elem_size=PADOUT,
                        queue_num=gcount % 4,
                    )
                    gcount += 1
                    k0 += kc
                nc.vector.tensor_add(
                    out=gat[:, :K2, :], in0=gat[:, :K2, :],
                    in1=gat[:, K2 : K2 + 1, :].to_broadcast([P, K2, PADOUT]),
                )
                nc.sync.dma_start(
                    out=hev[t * P : (t + 1) * P, :],
                    in_=gat[:, :K2, :OUT],
                )
            gatp_cm.__exit__(None, None, None)

            nc.scalar.dma_start(
                out=eidx[:, :].rearrange("(t p) k -> p t k", t=n_tiles, p=P),
                in_=eidx_all[:, :, :],
            )

    nc.compile()
    return nc


def _prepare_in_maps(X, V_embed, W_w, W_b):
    X = np.asarray(X, dtype=np.float32)
    V_embed = np.asarray(V_embed, dtype=np.float32)
    W_w = np.asarray(W_w, dtype=np.float32)
    W_b = np.asarray(W_b, dtype=np.float32)

    Xca = X[:, :, 1, :]  # [B, L, 3]
    in_maps = []
    for core in range(N_CORES):
        b, h = divmod(core, 2)
        in_maps.append({
            "xjt": np.ascontiguousarray(Xca[b].T),
            "xl": np.ascontiguousarray(Xca[b, h * ROWS_PER_CORE : (h + 1) * ROWS_PER_CORE]),
            "vt": np.ascontiguousarray(V_embed[b].T),
            "w1": np.ascontiguousarray(W_w[:C]),
            "w2": np.ascontiguousarray(W_w[C:]),
            "wb": np.ascontiguousarray(W_b.reshape(1, OUT)),
        })
    return in_maps


def _host_order_fixup(X, h32, e32):
    # Reorder each row's 32 candidates by (fp32 D, index) -- jax.lax.top_k's
    # exact ordering incl. sqrt-collapse ties -- and keep the first K.
    # D is recomputed bit-exactly: same elementwise fp32 ops as the reference.
    Xca = np.asarray(X, dtype=np.float32)[:, :, 1, :]  # [B, L, 3]
    h_EV = np.empty((B, L, K, OUT), np.float32)
    E_idx = np.empty((B, L, K), np.int32)
    for b in range(B):
        a = Xca[b]                      # [L, 3]
        cpts = Xca[b][e32[b]]           # [L, 32, 3]
        dx = (a[:, None, 0] - cpts[:, :, 0]).astype(np.float32)
        dy = (a[:, None, 1] - cpts[:, :, 1]).astype(np.float32)
        dz = (a[:, None, 2] - cpts[:, :, 2]).astype(np.float32)
        s = ((dx * dx + dy * dy) + dz * dz) + np.float32(1e-6)
        D = np.sqrt(s, dtype=np.float32)
        perm = np.lexsort((e32[b], D), axis=1)[:, :K]   # [L, K]
        E_idx[b] = np.take_along_axis(e32[b], perm, axis=1)
        h_EV[b] = np.take_along_axis(h32[b], perm[:, :, None], axis=1)
    return h_EV, E_idx


def _run(X, x_mask, V_embed, W_w, W_b, **run_kwargs):
    from concourse.bass_utils import run_bass_kernel_spmd

    if "nc" not in _CACHE:
        _CACHE["nc"] = _build_nc()
    nc = _CACHE["nc"]

    in_maps = _prepare_in_maps(X, V_embed, W_w, W_b)
    bkr = run_bass_kernel_spmd(nc, in_maps, list(range(N_CORES)), **run_kwargs)
    res = bkr.results

    h32 = np.empty((B, L, K2, OUT), np.float32)
    e32 = np.empty((B, L, K2), np.int64)
    for core in range(N_CORES):
        b, h = divmod(core, 2)
        rows = slice(h * ROWS_PER_CORE, (h + 1) * ROWS_PER_CORE)
        h32[b, rows] = res[core]["hev"].reshape(ROWS_PER_CORE, K2, OUT)
        e32[b, rows] = res[core]["eidx"].astype(np.int64)
    h_EV, E_idx = _host_order_fixup(X, h32, e32)
    return h_EV, E_idx, bkr


def kernel(X, x_mask, V_embed, W_w, W_b):
    h_EV, E_idx, _ = _run(X, x_mask, V_embed, W_w, W_b)
    return h_EV, E_idx
